# revision 1
# baseline (speedup 1.0000x reference)
"""BiLSTM Trainium2 kernel v2 (8 NeuronCores, SPMD, pipelined layers).

Roles (selected at runtime from partition id, same program on all cores):
  core 0: fwd layer-0    core 2: fwd layer-1
  core 1: bwd layer-0    core 3: bwd layer-1
  cores 4-7: spare (zero inputs, outputs ignored)

Single loop nest over NL superblock-steps; every loop body holds
[recurrence block of superblock t-1] + [x@Wx matmul block of superblock t]
so the bulk matmuls fill TensorE gaps left by the sequential recurrence.
Layer-0 output h0 is written in transposed (hT) layout and AllGather'd to
the partner layer-1 core once per superblock; the layer-1 core consumes it
LAG superblocks later (uniform collective placement keeps all cores issuing
identical collectives in identical order). The only role-divergent code is
the x-tile source (embedding gather vs h0 DMA), one state reset, and a
pid-derived output offset.

Gate order is permuted host-side to [g, i, f, o]; each gate group
accumulates (on top of an identity-matmul that pre-adds the precomputed
x@Wx+b term) into its own PSUM bank so tanh(g)/sigmoid(i,f) start while
the PE is still accumulating o. h stays transposed everywhere: the
recurrence's weight-stationary matmuls produce gates^T in PSUM at full
128-partition width, h^T feeds the next step directly, and layer-0's h^T
chunks are DMA'd straight to DRAM (no PE transposes anywhere).
"""

import numpy as np
import ml_dtypes

B = 16
H = 512
D = 1024
V = 32000
GATE = 4 * H
CH = 16            # recurrence steps per block
TOKB = CH * B      # tokens per block = 256
SB = 4             # blocks per superblock
SBTOK = SB * TOKB  # tokens per superblock = 1024
LAG = 3            # consumer lag in superblocks
PADB = 256         # hT_out block slots (pow2 so % is cheap); > NB + LAG*SB

_PROGRAM_CACHE = {}


def build_program(T):
    import concourse.mybir as mybir
    import concourse.tile as tile
    from concourse import bacc
    from concourse.bass import ds
    from concourse.masks import make_identity
    from concourse.tile_rust import add_dep_helper

    NT = T * B
    NB = NT // TOKB
    NSB = NB // SB
    NL = NSB + 1 + LAG

    f32 = mybir.dt.float32
    bf16 = mybir.dt.bfloat16
    i16 = mybir.dt.int16
    Sig = mybir.ActivationFunctionType.Sigmoid
    Tnh = mybir.ActivationFunctionType.Tanh
    Cpy = mybir.ActivationFunctionType.Identity

    nc = bacc.Bacc("TRN2", target_bir_lowering=False, debug=True, num_devices=8)

    tbl = nc.declare_dram_parameter("tbl", [V, D], bf16, isOutput=False)
    ids = nc.declare_dram_parameter("ids", [128, T], i16, isOutput=False)
    wx = nc.declare_dram_parameter("wx", [D, GATE], bf16, isOutput=False)
    wh = nc.declare_dram_parameter("wh", [H, GATE], bf16, isOutput=False)
    bt = nc.declare_dram_parameter("bt", [128, 16], f32, isOutput=False)
    hT_out = nc.declare_dram_parameter(
        "hT_out", [128, 4, PADB * TOKB], bf16, isOutput=True
    )

    with tile.TileContext(nc) as tc:
        with (
            tc.tile_pool(name="dram", bufs=1, space="DRAM") as dpool,
            tc.tile_pool(name="consts", bufs=1) as cpool,
            tc.tile_pool(name="xin", bufs=2) as xpool,
            tc.tile_pool(name="gxf", bufs=2) as gxpool,
            tc.tile_pool(name="gxc", bufs=2) as gcpool,
            tc.tile_pool(name="state", bufs=1) as spool,
            tc.tile_pool(name="tmp", bufs=3) as tpool,
            tc.tile_pool(name="hout", bufs=2) as hpool,
            tc.tile_pool(name="ps", bufs=2, space="PSUM") as pspool,
            tc.tile_pool(name="ps2", bufs=2, space="PSUM") as ps2pool,
            tc.tile_pool(name="ps3", bufs=2, space="PSUM") as ps3pool,
            tc.tile_pool(name="psx", bufs=2, space="PSUM") as psxpool,
        ):
            h0x = [dpool.tile([128, 4, SBTOK], bf16, tag=f"h0x{j}", name=f"h0x{j}") for j in range(NSB)]
            h0r = [
                dpool.tile([2, 128, 4, SBTOK], bf16, tag=f"h0r{j}", name=f"h0r{j}")
                for j in range(NSB)
            ]
            gxb = [
                dpool.tile([128, SB, CH, 16, B], bf16, tag=f"gxb{p}", name=f"gxb{p}")
                for p in range(2)
            ]

            pid = nc.partition_id()
            # 0 on layer-0 cores, LAG*SB on layer-1 cores (hT_out block offset)
            ofs_sv = nc.snap(((pid // 2) % 2) * (LAG * SB))

            wx_sb = cpool.tile([128, 8, GATE], bf16, tag="wx")
            nc.sync.dma_start(
                out=wx_sb, in_=wx[:, :].rearrange("(k p) m -> p k m", p=128)
            )
            wh_sb = cpool.tile([128, 4, GATE], bf16, tag="wh")
            nc.sync.dma_start(
                out=wh_sb, in_=wh[:, :].rearrange("(k p) m -> p k m", p=128)
            )
            bt_sb = cpool.tile([128, 16], f32, tag="bt")
            nc.sync.dma_start(out=bt_sb, in_=bt[:, :])
            ids_sb = cpool.tile([128, T], i16, tag="ids")
            nc.sync.dma_start(out=ids_sb, in_=ids[:, :])
            ident = cpool.tile([128, 128], bf16, tag="ident")
            make_identity(nc, ident)

            c_sb = spool.tile([128, 64], f32, tag="c")
            hT_sb = spool.tile([128, 4, 16], bf16, tag="h")
            nc.vector.memset(c_sb, 0.0)
            nc.vector.memset(hT_sb, 0.0)

            # zero the consumed half of the h0r buffers read during pipeline
            # warmup (before any AllGather has filled them)
            zt = cpool.tile([128, 4, SBTOK], bf16, tag="zt")
            nc.vector.memset(zt, 0.0)
            for j in range(min(LAG, NSB)):
                src = (j - LAG) % NSB
                nc.sync.dma_start(out=h0r[src][0], in_=zt)

            colls = {}
            xts = {}

            def emit_load(tt):
                """Stage the x-input tile for loop tt (issued two loops early)."""
                xt = xpool.tile([128, 2, 8, 512], bf16, tag="xt")
                xts[tt] = xt
                with tc.If((pid % 4) < 2) as cmp:
                    for g in range(2):
                        nc.gpsimd.dma_gather(
                            xt[:, g, :, :],
                            tbl[:, :],
                            ids_sb[:, ds(((tt % NSB) * SB) * CH + g * 32, 32)],
                            512,
                            512,
                            D,
                            transpose=True,
                        )
                with cmp.Else():
                    nc.vector.memset(xt[:, :, 4:8, :], 0.0)
                    src = (tt - LAG) % NSB
                    d = None
                    for g in range(2):
                        d = nc.sync.dma_start(
                            out=xt[:, g, 0:4, :],
                            in_=h0r[src][0][:, :, g * 512 : (g + 1) * 512],
                        )
                        if 0 <= tt - LAG < NSB and (tt - LAG) in colls:
                            add_dep_helper(
                                d.ins, colls[tt - LAG].ins, reason="xt after allgather"
                            )

            def emit_rec_body(t, i):
                """Recurrence for block i of superblock t-1."""
                sbi = (t - 1) % NSB
                gxc = gcpool.tile([128, CH, 16, B], bf16, tag="gxc")
                nc.sync.dma_start(out=gxc, in_=gxb[(t - 1) % 2][:, ds(i, 1), :, :, :])
                hTf = hpool.tile([128, 4, CH, 16], bf16, tag="hTf")
                for s in range(CH):
                    # three PSUM banks so early gate groups are readable while
                    # the PE is still accumulating later ones
                    psg_g = pspool.tile([128, 64], f32, tag="psg_g")
                    psg_if = ps2pool.tile([128, 128], f32, tag="psg_if")
                    psg_o = ps3pool.tile([128, 64], f32, tag="psg_o")
                    groups = [
                        (psg_g, 0, 4),
                        (psg_if, 4, 12),
                        (psg_o, 12, 16),
                    ]
                    for ptile, m0, m1 in groups:
                        nc.tensor.matmul(
                            ptile[:, :],
                            lhsT=ident,
                            rhs=gxc[:, s, m0:m1, :],
                            start=True,
                            stop=False,
                        )
                        for m in range(m0, m1):
                            for k in range(4):
                                rhs = hT_sb[:, k, :] if s == 0 else hTf[:, k, s - 1, :]
                                nc.tensor.matmul(
                                    ptile[:, (m - m0) * 16 : (m - m0 + 1) * 16],
                                    lhsT=wh_sb[:, k, m * 128 : (m + 1) * 128],
                                    rhs=rhs,
                                    start=False,
                                    stop=(m == m1 - 1 and k == 3),
                                )
                    tg = tpool.tile([128, 64], f32, tag="tg")
                    nc.scalar.activation(tg, psg_g[:, :], Tnh)
                    sif = tpool.tile([128, 128], f32, tag="sif")
                    nc.scalar.activation(sif, psg_if[:, :], Sig)
                    so = tpool.tile([128, 64], f32, tag="so")
                    nc.scalar.activation(so, psg_o[:, :], Sig)
                    ig = tpool.tile([128, 64], f32, tag="ig")
                    nc.vector.tensor_mul(ig, sif[:, 0:64], tg)
                    cf = tpool.tile([128, 64], f32, tag="cf")
                    nc.vector.tensor_mul(cf, c_sb, sif[:, 64:128])
                    nc.vector.tensor_add(c_sb, cf, ig)
                    th = tpool.tile([128, 64], f32, tag="th")
                    nc.scalar.activation(th, c_sb, Tnh)
                    nc.vector.tensor_mul(hTf[:, :, s, :], so, th)
                    if s == CH - 1:
                        nc.vector.tensor_copy(hT_sb, hTf[:, :, s, :])
                nc.sync.dma_start(
                    out=h0x[sbi][:, :, ds(i * TOKB, TOKB)], in_=hTf
                )
                goff = ((t - 1) * SB + PADB + i - ofs_sv) % PADB
                nc.sync.dma_start(
                    out=hT_out[:, :, ds(goff * TOKB, TOKB)], in_=hTf
                )

            def emit_xmm_body(t, i):
                """x @ Wx for block i of superblock t -> gxb[t % 2]."""
                xt = xts[t]
                gxf = gxpool.tile([128, CH, 16, B], bf16, tag="gxf")
                for m in range(16):
                    ps = psxpool.tile([128, TOKB], f32, tag="psx")
                    for k in range(8):
                        nc.tensor.matmul(
                            ps[:, :],
                            lhsT=wx_sb[:, k, m * 128 : (m + 1) * 128],
                            rhs=xt[:, ds(i // 2, 1), k, ds((i % 2) * TOKB, TOKB)],
                            start=(k == 0),
                            stop=(k == 7),
                        )
                    nc.scalar.activation(
                        gxf[:, :, m, :], ps[:, :], Cpy, bias=bt_sb[:, m : m + 1]
                    )
                nc.sync.dma_start(out=gxb[t % 2][:, ds(i, 1), :, :, :], in_=gxf)

            emit_load(0)
            emit_load(1)
            for t in range(NL):
                with tc.For_i(0, SB, 1) as i:
                    if t > 0:
                        emit_rec_body(t, i)
                    if t < NL - 1:
                        emit_xmm_body(t, i)
                j = t - 1
                if 0 <= j < NSB:
                    colls[j] = nc.gpsimd.collective_compute(
                        "AllGather",
                        mybir.AluOpType.bypass,
                        replica_groups=[[0, 2], [1, 3], [4, 6], [5, 7]],
                        ins=[h0x[j][:]],
                        outs=[h0r[j][:]],
                    )
                if t + 2 < NL:
                    emit_load(t + 2)
                if t == LAG:
                    # layer-1 cores start their real recurrence next loop
                    with tc.If((pid % 4) >= 2):
                        nc.vector.memset(c_sb, 0.0)
                        nc.vector.memset(hT_sb, 0.0)

    nc.finalize()
    return nc


def get_program(T):
    if T not in _PROGRAM_CACHE:
        _PROGRAM_CACHE[T] = build_program(T)
    return _PROGRAM_CACHE[T]


# gate reorder: reference layout [i, g, f, o] -> kernel layout [g, i, f, o]
_PERM = np.r_[512:1024, 0:512, 1024:1536, 1536:2048]


def _prep_weights(W, b, din):
    """W [din+H, 4H], b [4H] -> (wx [D,GATE] bf16, wh [H,GATE] bf16, bt [128,16] f32)."""
    bf = ml_dtypes.bfloat16
    W = np.asarray(W, np.float32)[:, _PERM]
    bv = np.asarray(b, np.float32)[_PERM].copy()
    bv[1024:1536] += 1.0  # haiku forget-gate +1 (f block now at 1024:1536)
    wxp = np.zeros((D, GATE), np.float32)
    wxp[0:din] = W[0:din]
    whp = np.ascontiguousarray(W[din : din + H])
    btp = np.ascontiguousarray(bv.reshape(16, 128).T)
    return wxp.astype(bf), whp.astype(bf), btp


def _wrap_ids(ids2d):
    return np.tile(np.asarray(ids2d).astype(np.int16), (8, 1))


def make_in_maps(input_ids, embed_table, fwd_W0, fwd_b0, fwd_W1, fwd_b1,
                 bwd_W0, bwd_b0, bwd_W1, bwd_b1):
    T = input_ids.shape[1]
    bf = ml_dtypes.bfloat16
    tbl = np.ascontiguousarray(np.asarray(embed_table, np.float32)).astype(bf)

    ids_f = _wrap_ids(input_ids)
    ids_b = _wrap_ids(np.asarray(input_ids)[:, ::-1])

    z = np.zeros
    base = dict(
        tbl=z((V, D), bf),
        ids=z((128, T), np.int16),
        wx=z((D, GATE), bf),
        wh=z((H, GATE), bf),
        bt=z((128, 16), np.float32),
    )
    maps = [dict(base) for _ in range(8)]

    fx0, fh0, fb0t = _prep_weights(fwd_W0, fwd_b0, D)
    bx0, bh0, bb0t = _prep_weights(bwd_W0, bwd_b0, D)
    fx1, fh1, fb1t = _prep_weights(fwd_W1, fwd_b1, H)
    bx1, bh1, bb1t = _prep_weights(bwd_W1, bwd_b1, H)

    maps[0].update(tbl=tbl, ids=ids_f, wx=fx0, wh=fh0, bt=fb0t)
    maps[1].update(tbl=tbl, ids=ids_b, wx=bx0, wh=bh0, bt=bb0t)
    maps[2].update(wx=fx1, wh=fh1, bt=fb1t)
    maps[3].update(wx=bx1, wh=bh1, bt=bb1t)
    return maps


def assemble_output(hT_fwd, hT_bwd, T):
    def unT(a):
        arr = np.asarray(a, np.float32)[:, :, : T * 16].reshape(128, 4, T, 16)
        return np.ascontiguousarray(arr.transpose(3, 2, 1, 0).reshape(16, T, 512))

    F = unT(hT_fwd)
    Bo = unT(hT_bwd)[:, ::-1, :]
    return np.ascontiguousarray(np.concatenate([F, Bo], axis=2))


def kernel(**inputs):
    from concourse.bass_utils import run_bass_kernel_spmd

    input_ids = np.asarray(inputs["input_ids"])
    T = input_ids.shape[1]
    nc = get_program(T)
    maps = make_in_maps(**inputs)
    res = run_bass_kernel_spmd(nc, maps, list(range(8)))
    return assemble_output(
        res.results[2]["hT_out"], res.results[3]["hT_out"], T
    )



# revision 2
# speedup vs baseline: 1.7459x; 1.7459x over previous
"""BiLSTM Trainium2 kernel v3 (8 NeuronCores, SPMD, pipelined layers).

Roles (selected at runtime from partition id, same program on all cores):
  core 0: fwd layer-0    core 2: fwd layer-1
  core 1: bwd layer-0    core 3: bwd layer-1
  cores 4-7: spare (zero inputs, outputs ignored)

v3 vs v2: the per-superblock SB loop is fully unrolled in Python (no
tc.For_i hardware loop). With the hardware loop, every x@Wx / ident
matmul source was register-indirect, costing ~372ns each on the PE
queue (register-AP setup ops serialize with the matmuls); unrolled,
all addresses are compile-time static so the bulk matmuls pipeline at
streaming rate. x@Wx also runs at N=512 (two half-superblock tiles)
instead of N=256, and the three ident pre-add matmuls of each
recurrence step are emitted adjacently so their LDWEIGHTS pipeline.

Gate order is permuted host-side to [g, i, f, o]; each gate group
accumulates (on top of an identity-matmul that pre-adds the precomputed
x@Wx+b term) into its own PSUM bank so tanh(g)/sigmoid(i,f) start while
the PE is still accumulating o. h stays transposed everywhere: the
recurrence's weight-stationary matmuls produce gates^T in PSUM at full
128-partition width, h^T feeds the next step directly, and layer-0's h^T
chunks are DMA'd straight to DRAM (no PE transposes anywhere).
"""

import numpy as np
import ml_dtypes

B = 16
H = 512
D = 1024
V = 32000
GATE = 4 * H
CH = 16            # recurrence steps per block
TOKB = CH * B      # tokens per block = 256
SB = 4             # blocks per superblock
SBTOK = SB * TOKB  # tokens per superblock = 1024
LAG = 3            # consumer lag in superblocks
PADB = 256         # hT_out block slots (pow2 so % is cheap); > NB + LAG*SB

_PROGRAM_CACHE = {}


def build_program(T):
    import concourse.mybir as mybir
    import concourse.tile as tile
    from concourse import bacc
    from concourse.bass import ds
    from concourse.masks import make_identity
    from concourse.tile_rust import add_dep_helper

    NT = T * B
    NB = NT // TOKB
    NSB = NB // SB
    NL = NSB + 1 + LAG

    f32 = mybir.dt.float32
    bf16 = mybir.dt.bfloat16
    i16 = mybir.dt.int16
    Sig = mybir.ActivationFunctionType.Sigmoid
    Tnh = mybir.ActivationFunctionType.Tanh
    Cpy = mybir.ActivationFunctionType.Identity

    nc = bacc.Bacc("TRN2", target_bir_lowering=False, debug=True, num_devices=8)

    tbl = nc.declare_dram_parameter("tbl", [V, D], bf16, isOutput=False)
    ids = nc.declare_dram_parameter("ids", [128, T], i16, isOutput=False)
    wx = nc.declare_dram_parameter("wx", [D, GATE], bf16, isOutput=False)
    wh = nc.declare_dram_parameter("wh", [H, GATE], bf16, isOutput=False)
    bt = nc.declare_dram_parameter("bt", [128, 16], f32, isOutput=False)
    hT_out = nc.declare_dram_parameter(
        "hT_out", [128, 4, PADB * TOKB], bf16, isOutput=True
    )

    with tile.TileContext(nc) as tc:
        with (
            tc.tile_pool(name="dram", bufs=1, space="DRAM") as dpool,
            tc.tile_pool(name="consts", bufs=1) as cpool,
            tc.tile_pool(name="xin", bufs=2) as xpool,
            tc.tile_pool(name="gxf", bufs=2) as gxpool,
            tc.tile_pool(name="gxc", bufs=2) as gcpool,
            tc.tile_pool(name="state", bufs=1) as spool,
            tc.tile_pool(name="tmp", bufs=3) as tpool,
            tc.tile_pool(name="hout", bufs=2) as hpool,
            tc.tile_pool(name="ps", bufs=2, space="PSUM") as pspool,
            tc.tile_pool(name="ps2", bufs=2, space="PSUM") as ps2pool,
            tc.tile_pool(name="ps3", bufs=2, space="PSUM") as ps3pool,
            tc.tile_pool(name="psx", bufs=2, space="PSUM") as psxpool,
        ):
            h0x = [dpool.tile([128, 4, SBTOK], bf16, tag=f"h0x{j}", name=f"h0x{j}") for j in range(NSB)]
            h0r = [
                dpool.tile([2, 128, 4, SBTOK], bf16, tag=f"h0r{j}", name=f"h0r{j}")
                for j in range(NSB)
            ]
            gxb = [
                dpool.tile([128, SB, CH, 16, B], bf16, tag=f"gxb{p}", name=f"gxb{p}")
                for p in range(2)
            ]

            pid = nc.partition_id()
            # 0 on layer-0 cores, LAG*SB on layer-1 cores (hT_out block offset)
            ofs_sv = nc.snap(((pid // 2) % 2) * (LAG * SB))

            wx_sb = cpool.tile([128, 8, GATE], bf16, tag="wx")
            nc.sync.dma_start(
                out=wx_sb, in_=wx[:, :].rearrange("(k p) m -> p k m", p=128)
            )
            wh_sb = cpool.tile([128, 4, GATE], bf16, tag="wh")
            nc.sync.dma_start(
                out=wh_sb, in_=wh[:, :].rearrange("(k p) m -> p k m", p=128)
            )
            bt_sb = cpool.tile([128, 16], f32, tag="bt")
            nc.sync.dma_start(out=bt_sb, in_=bt[:, :])
            ids_sb = cpool.tile([128, T], i16, tag="ids")
            nc.sync.dma_start(out=ids_sb, in_=ids[:, :])
            ident = cpool.tile([128, 128], bf16, tag="ident")
            make_identity(nc, ident)

            c_sb = spool.tile([128, 64], f32, tag="c")
            hT_sb = spool.tile([128, 4, 16], bf16, tag="h")
            nc.vector.memset(c_sb, 0.0)
            nc.vector.memset(hT_sb, 0.0)

            # zero the consumed half of the h0r buffers read during pipeline
            # warmup (before any AllGather has filled them)
            zt = cpool.tile([128, 4, SBTOK], bf16, tag="zt")
            nc.vector.memset(zt, 0.0)
            for j in range(min(LAG, NSB)):
                src = (j - LAG) % NSB
                nc.sync.dma_start(out=h0r[src][0], in_=zt)

            colls = {}
            xts = {}

            def emit_load(tt):
                """Stage the x-input tile for loop tt (issued two loops early)."""
                xt = xpool.tile([128, 2, 8, 512], bf16, tag="xt")
                xts[tt] = xt
                with tc.If((pid % 4) < 2) as cmp:
                    for g in range(2):
                        nc.gpsimd.dma_gather(
                            xt[:, g, :, :],
                            tbl[:, :],
                            ids_sb[:, ds(((tt % NSB) * SB) * CH + g * 32, 32)],
                            512,
                            512,
                            D,
                            transpose=True,
                        )
                with cmp.Else():
                    nc.vector.memset(xt[:, :, 4:8, :], 0.0)
                    src = (tt - LAG) % NSB
                    d = None
                    for g in range(2):
                        d = nc.sync.dma_start(
                            out=xt[:, g, 0:4, :],
                            in_=h0r[src][0][:, :, g * 512 : (g + 1) * 512],
                        )
                        if 0 <= tt - LAG < NSB and (tt - LAG) in colls:
                            add_dep_helper(
                                d.ins, colls[tt - LAG].ins, reason="xt after allgather"
                            )

            def emit_rec_body(t, i):
                """Recurrence for block i of superblock t-1."""
                sbi = (t - 1) % NSB
                gxc = gcpool.tile([128, CH, 16, B], bf16, tag="gxc")
                nc.sync.dma_start(out=gxc, in_=gxb[(t - 1) % 2][:, ds(i, 1), :, :, :])
                hTf = hpool.tile([128, 4, CH, 16], bf16, tag="hTf")
                for s in range(CH):
                    # three PSUM banks so early gate groups are readable while
                    # the PE is still accumulating later ones
                    psg_g = pspool.tile([128, 64], f32, tag="psg_g")
                    psg_if = ps2pool.tile([128, 128], f32, tag="psg_if")
                    psg_o = ps3pool.tile([128, 64], f32, tag="psg_o")
                    groups = [
                        (psg_g, 0, 4),
                        (psg_if, 4, 12),
                        (psg_o, 12, 16),
                    ]
                    # ident pre-adds first (adjacent: shared stationary operand)
                    for ptile, m0, m1 in groups:
                        nc.tensor.matmul(
                            ptile[:, :],
                            lhsT=ident,
                            rhs=gxc[:, s, m0:m1, :],
                            start=True,
                            stop=False,
                        )
                    for ptile, m0, m1 in groups:
                        for m in range(m0, m1):
                            for k in range(4):
                                rhs = hT_sb[:, k, :] if s == 0 else hTf[:, k, s - 1, :]
                                nc.tensor.matmul(
                                    ptile[:, (m - m0) * 16 : (m - m0 + 1) * 16],
                                    lhsT=wh_sb[:, k, m * 128 : (m + 1) * 128],
                                    rhs=rhs,
                                    start=False,
                                    stop=(m == m1 - 1 and k == 3),
                                )
                    tg = tpool.tile([128, 64], f32, tag="tg")
                    nc.scalar.activation(tg, psg_g[:, :], Tnh)
                    sif = tpool.tile([128, 128], f32, tag="sif")
                    nc.scalar.activation(sif, psg_if[:, :], Sig)
                    so = tpool.tile([128, 64], f32, tag="so")
                    nc.scalar.activation(so, psg_o[:, :], Sig)
                    ig = tpool.tile([128, 64], f32, tag="ig")
                    nc.vector.tensor_mul(ig, sif[:, 0:64], tg)
                    cf = tpool.tile([128, 64], f32, tag="cf")
                    nc.vector.tensor_mul(cf, c_sb, sif[:, 64:128])
                    nc.vector.tensor_add(c_sb, cf, ig)
                    th = tpool.tile([128, 64], f32, tag="th")
                    nc.scalar.activation(th, c_sb, Tnh)
                    nc.vector.tensor_mul(hTf[:, :, s, :], so, th)
                    if s == CH - 1:
                        nc.vector.tensor_copy(hT_sb, hTf[:, :, s, :])
                nc.sync.dma_start(
                    out=h0x[sbi][:, :, ds(i * TOKB, TOKB)], in_=hTf
                )
                goff = ((t - 1) * SB + PADB + i - ofs_sv) % PADB
                nc.sync.dma_start(
                    out=hT_out[:, :, ds(goff * TOKB, TOKB)], in_=hTf
                )

            def emit_xmm_half(t, g):
                """x @ Wx for token-half g (512 tokens = blocks 2g,2g+1) of
                superblock t -> gxb[t % 2]."""
                xt = xts[t]
                gxf = gxpool.tile([128, 2, CH, 16, B], bf16, tag="gxf")
                for m in range(16):
                    ps = psxpool.tile([128, 512], f32, tag="psx")
                    for k in range(8):
                        nc.tensor.matmul(
                            ps[:, :],
                            lhsT=wx_sb[:, k, m * 128 : (m + 1) * 128],
                            rhs=xt[:, g, k, :],
                            start=(k == 0),
                            stop=(k == 7),
                        )
                    nc.scalar.activation(
                        gxf[:, :, :, m, :],
                        ps[:, :].rearrange("p (b s c) -> p b s c", b=2, s=CH),
                        Cpy,
                        bias=bt_sb[:, m : m + 1],
                    )
                nc.sync.dma_start(
                    out=gxb[t % 2][:, 2 * g : 2 * g + 2, :, :, :], in_=gxf
                )

            emit_load(0)
            emit_load(1)
            for t in range(NL):
                for i in range(SB):
                    if t > 0:
                        emit_rec_body(t, i)
                    if t < NL - 1 and i < 2:
                        emit_xmm_half(t, i)
                j = t - 1
                if 0 <= j < NSB:
                    colls[j] = nc.gpsimd.collective_compute(
                        "AllGather",
                        mybir.AluOpType.bypass,
                        replica_groups=[[0, 2], [1, 3], [4, 6], [5, 7]],
                        ins=[h0x[j][:]],
                        outs=[h0r[j][:]],
                    )
                if t + 2 < NL:
                    emit_load(t + 2)
                if t == LAG:
                    # layer-1 cores start their real recurrence next loop
                    with tc.If((pid % 4) >= 2):
                        nc.vector.memset(c_sb, 0.0)
                        nc.vector.memset(hT_sb, 0.0)

    nc.finalize()
    return nc


def get_program(T):
    if T not in _PROGRAM_CACHE:
        _PROGRAM_CACHE[T] = build_program(T)
    return _PROGRAM_CACHE[T]


# gate reorder: reference layout [i, g, f, o] -> kernel layout [g, i, f, o]
_PERM = np.r_[512:1024, 0:512, 1024:1536, 1536:2048]


def _prep_weights(W, b, din):
    """W [din+H, 4H], b [4H] -> (wx [D,GATE] bf16, wh [H,GATE] bf16, bt [128,16] f32)."""
    bf = ml_dtypes.bfloat16
    W = np.asarray(W, np.float32)[:, _PERM]
    bv = np.asarray(b, np.float32)[_PERM].copy()
    bv[1024:1536] += 1.0  # haiku forget-gate +1 (f block now at 1024:1536)
    wxp = np.zeros((D, GATE), np.float32)
    wxp[0:din] = W[0:din]
    whp = np.ascontiguousarray(W[din : din + H])
    btp = np.ascontiguousarray(bv.reshape(16, 128).T)
    return wxp.astype(bf), whp.astype(bf), btp


def _wrap_ids(ids2d):
    return np.tile(np.asarray(ids2d).astype(np.int16), (8, 1))


def make_in_maps(input_ids, embed_table, fwd_W0, fwd_b0, fwd_W1, fwd_b1,
                 bwd_W0, bwd_b0, bwd_W1, bwd_b1):
    T = input_ids.shape[1]
    bf = ml_dtypes.bfloat16
    tbl = np.ascontiguousarray(np.asarray(embed_table, np.float32)).astype(bf)

    ids_f = _wrap_ids(input_ids)
    ids_b = _wrap_ids(np.asarray(input_ids)[:, ::-1])

    z = np.zeros
    base = dict(
        tbl=z((V, D), bf),
        ids=z((128, T), np.int16),
        wx=z((D, GATE), bf),
        wh=z((H, GATE), bf),
        bt=z((128, 16), np.float32),
    )
    maps = [dict(base) for _ in range(8)]

    fx0, fh0, fb0t = _prep_weights(fwd_W0, fwd_b0, D)
    bx0, bh0, bb0t = _prep_weights(bwd_W0, bwd_b0, D)
    fx1, fh1, fb1t = _prep_weights(fwd_W1, fwd_b1, H)
    bx1, bh1, bb1t = _prep_weights(bwd_W1, bwd_b1, H)

    maps[0].update(tbl=tbl, ids=ids_f, wx=fx0, wh=fh0, bt=fb0t)
    maps[1].update(tbl=tbl, ids=ids_b, wx=bx0, wh=bh0, bt=bb0t)
    maps[2].update(wx=fx1, wh=fh1, bt=fb1t)
    maps[3].update(wx=bx1, wh=bh1, bt=bb1t)
    return maps


def assemble_output(hT_fwd, hT_bwd, T):
    def unT(a):
        arr = np.asarray(a, np.float32)[:, :, : T * 16].reshape(128, 4, T, 16)
        return np.ascontiguousarray(arr.transpose(3, 2, 1, 0).reshape(16, T, 512))

    F = unT(hT_fwd)
    Bo = unT(hT_bwd)[:, ::-1, :]
    return np.ascontiguousarray(np.concatenate([F, Bo], axis=2))


def kernel(**inputs):
    from concourse.bass_utils import run_bass_kernel_spmd

    input_ids = np.asarray(inputs["input_ids"])
    T = input_ids.shape[1]
    nc = get_program(T)
    maps = make_in_maps(**inputs)
    res = run_bass_kernel_spmd(nc, maps, list(range(8)))
    return assemble_output(
        res.results[2]["hT_out"], res.results[3]["hT_out"], T
    )


# revision 4
# speedup vs baseline: 1.7907x; 1.0257x over previous
"""BiLSTM Trainium2 kernel v4 (8 NeuronCores, SPMD, pipelined layers).

Roles (selected at runtime from partition id, same program on all cores):
  core 0: fwd layer-0    core 2: fwd layer-1
  core 1: bwd layer-0    core 3: bwd layer-1
  cores 4-7: spare (zero inputs, outputs ignored)

v4 vs v3: the per-step critical chain is restructured around the
engine FIFOs. ScalarE runs exactly the four chain ACTs per step
(tanh g, sigmoid if, sigmoid o, tanh c); the x@Wx bias-add/copy moved
to VectorE (tensor_scalar_add), so bulk-matmul PSUM banks are freed
promptly instead of queueing behind chain ACTs. The cell state c lives
in the right half of a [128,128] tile G whose left half receives
tanh(g), so i*tanh(g) and f*c fuse into ONE VectorE multiply
(igcf = sif * G) followed by one add. x@Wx matmuls are spread across
the recurrence steps (4 N=512 matmuls after each step's o-group) so
they execute inside the PE idle window while the boundary chain runs.

Gate order is permuted host-side to [g, i, f, o]; each gate group
accumulates (on top of an identity-matmul that pre-adds the precomputed
x@Wx+b term) into its own PSUM bank so tanh(g)/sigmoid(i,f) start while
the PE is still accumulating o. h stays transposed everywhere: the
recurrence's weight-stationary matmuls produce gates^T in PSUM at full
128-partition width, h^T feeds the next step directly, and layer-0's h^T
chunks are DMA'd straight to DRAM (no PE transposes anywhere).
"""

import numpy as np
import ml_dtypes

B = 16
H = 512
D = 1024
V = 32000
GATE = 4 * H
CH = 16            # recurrence steps per block
TOKB = CH * B      # tokens per block = 256
SB = 4             # blocks per superblock
SBTOK = SB * TOKB  # tokens per superblock = 1024
LAG = 3            # consumer lag in superblocks
PADB = 256         # hT_out block slots (pow2 so % is cheap); > NB + LAG*SB

_PROGRAM_CACHE = {}


def build_program(T):
    import concourse.mybir as mybir
    import concourse.tile as tile
    from concourse import bacc
    from concourse.bass import ds
    from concourse.masks import make_identity
    from concourse.tile_rust import add_dep_helper

    NT = T * B
    NB = NT // TOKB
    NSB = NB // SB
    NL = NSB + 1 + LAG

    f32 = mybir.dt.float32
    bf16 = mybir.dt.bfloat16
    i16 = mybir.dt.int16
    Sig = mybir.ActivationFunctionType.Sigmoid
    Tnh = mybir.ActivationFunctionType.Tanh

    nc = bacc.Bacc("TRN2", target_bir_lowering=False, debug=True, num_devices=8)

    tbl = nc.declare_dram_parameter("tbl", [V, D], bf16, isOutput=False)
    ids = nc.declare_dram_parameter("ids", [128, T], i16, isOutput=False)
    wx = nc.declare_dram_parameter("wx", [D, GATE], bf16, isOutput=False)
    wh = nc.declare_dram_parameter("wh", [H, GATE], bf16, isOutput=False)
    bt = nc.declare_dram_parameter("bt", [128, 16], f32, isOutput=False)
    hT_out = nc.declare_dram_parameter(
        "hT_out", [128, 4, PADB * TOKB], bf16, isOutput=True
    )

    with tile.TileContext(nc) as tc:
        with (
            tc.tile_pool(name="dram", bufs=1, space="DRAM") as dpool,
            tc.tile_pool(name="consts", bufs=1) as cpool,
            tc.tile_pool(name="xin", bufs=2) as xpool,
            tc.tile_pool(name="gxf", bufs=2) as gxpool,
            tc.tile_pool(name="gxc", bufs=2) as gcpool,
            tc.tile_pool(name="state", bufs=1) as spool,
            tc.tile_pool(name="tmp", bufs=3) as tpool,
            tc.tile_pool(name="hout", bufs=2) as hpool,
            tc.tile_pool(name="ps", bufs=2, space="PSUM") as pspool,
            tc.tile_pool(name="ps2", bufs=2, space="PSUM") as ps2pool,
            tc.tile_pool(name="ps3", bufs=2, space="PSUM") as ps3pool,
            tc.tile_pool(name="psx", bufs=2, space="PSUM") as psxpool,
        ):
            h0x = [dpool.tile([128, 4, SBTOK], bf16, tag=f"h0x{j}", name=f"h0x{j}") for j in range(NSB)]
            h0r = [
                dpool.tile([2, 128, 4, SBTOK], bf16, tag=f"h0r{j}", name=f"h0r{j}")
                for j in range(NSB)
            ]
            gxb = [
                dpool.tile([128, SB, CH, 16, B], bf16, tag=f"gxb{p}", name=f"gxb{p}")
                for p in range(2)
            ]

            pid = nc.partition_id()
            # 0 on layer-0 cores, LAG*SB on layer-1 cores (hT_out block offset)
            ofs_sv = nc.snap(((pid // 2) % 2) * (LAG * SB))

            wx_sb = cpool.tile([128, 8, GATE], bf16, tag="wx")
            nc.sync.dma_start(
                out=wx_sb, in_=wx[:, :].rearrange("(k p) m -> p k m", p=128)
            )
            wh_sb = cpool.tile([128, 4, GATE], bf16, tag="wh")
            nc.sync.dma_start(
                out=wh_sb, in_=wh[:, :].rearrange("(k p) m -> p k m", p=128)
            )
            bt_sb = cpool.tile([128, 16], f32, tag="bt")
            nc.sync.dma_start(out=bt_sb, in_=bt[:, :])
            ids_sb = cpool.tile([128, T], i16, tag="ids")
            nc.sync.dma_start(out=ids_sb, in_=ids[:, :])
            ident = cpool.tile([128, 128], bf16, tag="ident")
            make_identity(nc, ident)

            # G: [tanh(g) scratch | cell state c]
            G_sb = spool.tile([128, 128], f32, tag="G")
            hT_sb = spool.tile([128, 4, 16], bf16, tag="h")
            nc.vector.memset(G_sb, 0.0)
            nc.vector.memset(hT_sb, 0.0)

            # zero the consumed half of the h0r buffers read during pipeline
            # warmup (before any AllGather has filled them)
            zt = cpool.tile([128, 4, SBTOK], bf16, tag="zt")
            nc.vector.memset(zt, 0.0)
            for j in range(min(LAG, NSB)):
                src = (j - LAG) % NSB
                nc.sync.dma_start(out=h0r[src][0], in_=zt)

            colls = {}
            xts = {}
            xmm_state = {}  # t -> [psx, gxf]

            def emit_load(tt):
                """Stage the x-input tile for loop tt (issued two loops early)."""
                xt = xpool.tile([128, 2, 8, 512], bf16, tag="xt")
                xts[tt] = xt
                with tc.If((pid % 4) < 2) as cmp:
                    for g in range(2):
                        nc.gpsimd.dma_gather(
                            xt[:, g, :, :],
                            tbl[:, :],
                            ids_sb[:, ds(((tt % NSB) * SB) * CH + g * 32, 32)],
                            512,
                            512,
                            D,
                            transpose=True,
                        )
                with cmp.Else():
                    nc.vector.memset(xt[:, :, 4:8, :], 0.0)
                    src = (tt - LAG) % NSB
                    d = None
                    for g in range(2):
                        d = nc.sync.dma_start(
                            out=xt[:, g, 0:4, :],
                            in_=h0r[src][0][:, :, g * 512 : (g + 1) * 512],
                        )
                        if 0 <= tt - LAG < NSB and (tt - LAG) in colls:
                            add_dep_helper(
                                d.ins, colls[tt - LAG].ins, reason="xt after allgather"
                            )

            def emit_xmm_piece(t, hs):
                """Half an m-tile (4 of 8 k-matmuls) of x @ Wx for superblock t.
                hs in 0..63 indexes (token-half g, m-tile, k-phase)."""
                g = hs // 32
                m = (hs % 32) // 2
                phase = hs % 2
                xt = xts[t]
                st = xmm_state.setdefault(t, [None, None])
                if hs % 32 == 0:
                    st[1] = gxpool.tile(
                        [128, 2, CH, 16, B], bf16, tag="gxf", name="gxf"
                    )
                if phase == 0:
                    st[0] = psxpool.tile([128, 512], f32, tag="psx", name="psx")
                ps, gxf = st[0], st[1]
                for k in range(4 * phase, 4 * phase + 4):
                    nc.tensor.matmul(
                        ps[:, :],
                        lhsT=wx_sb[:, k, m * 128 : (m + 1) * 128],
                        rhs=xt[:, g, k, :],
                        start=(k == 0),
                        stop=(k == 7),
                    )
                if phase == 1:
                    # bias-add + PSUM->SBUF copy on VectorE (keeps ScalarE for
                    # the recurrence chain ACTs)
                    nc.vector.tensor_scalar_add(
                        gxf[:, :, :, m, :],
                        ps[:, :].rearrange("p (b s c) -> p b s c", b=2, s=CH),
                        bt_sb[:, m : m + 1],
                    )
                    if m == 15:
                        nc.sync.dma_start(
                            out=gxb[t % 2][:, 2 * g : 2 * g + 2, :, :, :], in_=gxf
                        )

            def emit_rec_body(t, i, with_xmm):
                """Recurrence for block i of superblock t-1 (+ interleaved
                x@Wx pieces of superblock t)."""
                sbi = (t - 1) % NSB
                gxc = gcpool.tile([128, CH, 16, B], bf16, tag="gxc")
                nc.sync.dma_start(out=gxc, in_=gxb[(t - 1) % 2][:, ds(i, 1), :, :, :])
                hTf = hpool.tile([128, 4, CH, 16], bf16, tag="hTf")
                for s in range(CH):
                    # three PSUM banks so early gate groups are readable while
                    # the PE is still accumulating later ones
                    psg_g = pspool.tile([128, 64], f32, tag="psg_g")
                    psg_if = ps2pool.tile([128, 128], f32, tag="psg_if")
                    psg_o = ps3pool.tile([128, 64], f32, tag="psg_o")
                    groups = [
                        (psg_g, 0, 4),
                        (psg_if, 4, 12),
                        (psg_o, 12, 16),
                    ]
                    # ident pre-adds first (adjacent: shared stationary operand)
                    for ptile, m0, m1 in groups:
                        nc.tensor.matmul(
                            ptile[:, :],
                            lhsT=ident,
                            rhs=gxc[:, s, m0:m1, :],
                            start=True,
                            stop=False,
                        )
                    for ptile, m0, m1 in groups:
                        for m in range(m0, m1):
                            for k in range(4):
                                rhs = hT_sb[:, k, :] if s == 0 else hTf[:, k, s - 1, :]
                                nc.tensor.matmul(
                                    ptile[:, (m - m0) * 16 : (m - m0 + 1) * 16],
                                    lhsT=wh_sb[:, k, m * 128 : (m + 1) * 128],
                                    rhs=rhs,
                                    start=False,
                                    stop=(m == m1 - 1 and k == 3),
                                )
                    # PE: x@Wx piece fills the boundary window after o-group
                    if with_xmm:
                        emit_xmm_piece(t, i * CH + s)
                    # ScalarE chain: tg, sif, so, th; VectorE: igcf, cadd, h
                    nc.scalar.activation(G_sb[:, 0:64], psg_g[:, :], Tnh)
                    sif = tpool.tile([128, 128], f32, tag="sif")
                    nc.scalar.activation(sif, psg_if[:, :], Sig)
                    so = tpool.tile([128, 64], f32, tag="so")
                    nc.scalar.activation(so, psg_o[:, :], Sig)
                    igcf = tpool.tile([128, 128], f32, tag="igcf")
                    nc.vector.tensor_mul(igcf, sif, G_sb)
                    nc.vector.tensor_add(G_sb[:, 64:128], igcf[:, 0:64], igcf[:, 64:128])
                    th = tpool.tile([128, 64], f32, tag="th")
                    nc.scalar.activation(th, G_sb[:, 64:128], Tnh)
                    nc.vector.tensor_mul(hTf[:, :, s, :], so, th)
                    if s == CH - 1:
                        nc.vector.tensor_copy(hT_sb, hTf[:, :, s, :])
                nc.sync.dma_start(
                    out=h0x[sbi][:, :, ds(i * TOKB, TOKB)], in_=hTf
                )
                goff = ((t - 1) * SB + PADB + i - ofs_sv) % PADB
                nc.sync.dma_start(
                    out=hT_out[:, :, ds(goff * TOKB, TOKB)], in_=hTf
                )

            emit_load(0)
            emit_load(1)
            for t in range(NL):
                with_xmm = t < NL - 1
                for i in range(SB):
                    if t > 0:
                        emit_rec_body(t, i, with_xmm)
                    elif with_xmm:
                        for s in range(CH):
                            emit_xmm_piece(t, i * CH + s)
                j = t - 1
                if 0 <= j < NSB:
                    colls[j] = nc.gpsimd.collective_compute(
                        "AllGather",
                        mybir.AluOpType.bypass,
                        replica_groups=[[0, 2], [1, 3], [4, 6], [5, 7]],
                        ins=[h0x[j][:]],
                        outs=[h0r[j][:]],
                    )
                if t + 2 < NL:
                    emit_load(t + 2)
                if t == LAG:
                    # layer-1 cores start their real recurrence next loop
                    with tc.If((pid % 4) >= 2):
                        nc.vector.memset(G_sb, 0.0)
                        nc.vector.memset(hT_sb, 0.0)

    nc.finalize()
    return nc


def get_program(T):
    if T not in _PROGRAM_CACHE:
        _PROGRAM_CACHE[T] = build_program(T)
    return _PROGRAM_CACHE[T]


# gate reorder: reference layout [i, g, f, o] -> kernel layout [g, i, f, o]
_PERM = np.r_[512:1024, 0:512, 1024:1536, 1536:2048]


def _prep_weights(W, b, din):
    """W [din+H, 4H], b [4H] -> (wx [D,GATE] bf16, wh [H,GATE] bf16, bt [128,16] f32)."""
    bf = ml_dtypes.bfloat16
    W = np.asarray(W, np.float32)[:, _PERM]
    bv = np.asarray(b, np.float32)[_PERM].copy()
    bv[1024:1536] += 1.0  # haiku forget-gate +1 (f block now at 1024:1536)
    wxp = np.zeros((D, GATE), np.float32)
    wxp[0:din] = W[0:din]
    whp = np.ascontiguousarray(W[din : din + H])
    btp = np.ascontiguousarray(bv.reshape(16, 128).T)
    return wxp.astype(bf), whp.astype(bf), btp


def _wrap_ids(ids2d):
    return np.tile(np.asarray(ids2d).astype(np.int16), (8, 1))


def make_in_maps(input_ids, embed_table, fwd_W0, fwd_b0, fwd_W1, fwd_b1,
                 bwd_W0, bwd_b0, bwd_W1, bwd_b1):
    T = input_ids.shape[1]
    bf = ml_dtypes.bfloat16
    tbl = np.ascontiguousarray(np.asarray(embed_table, np.float32)).astype(bf)

    ids_f = _wrap_ids(input_ids)
    ids_b = _wrap_ids(np.asarray(input_ids)[:, ::-1])

    z = np.zeros
    base = dict(
        tbl=z((V, D), bf),
        ids=z((128, T), np.int16),
        wx=z((D, GATE), bf),
        wh=z((H, GATE), bf),
        bt=z((128, 16), np.float32),
    )
    maps = [dict(base) for _ in range(8)]

    fx0, fh0, fb0t = _prep_weights(fwd_W0, fwd_b0, D)
    bx0, bh0, bb0t = _prep_weights(bwd_W0, bwd_b0, D)
    fx1, fh1, fb1t = _prep_weights(fwd_W1, fwd_b1, H)
    bx1, bh1, bb1t = _prep_weights(bwd_W1, bwd_b1, H)

    maps[0].update(tbl=tbl, ids=ids_f, wx=fx0, wh=fh0, bt=fb0t)
    maps[1].update(tbl=tbl, ids=ids_b, wx=bx0, wh=bh0, bt=bb0t)
    maps[2].update(wx=fx1, wh=fh1, bt=fb1t)
    maps[3].update(wx=bx1, wh=bh1, bt=bb1t)
    return maps


def assemble_output(hT_fwd, hT_bwd, T):
    def unT(a):
        arr = np.asarray(a, np.float32)[:, :, : T * 16].reshape(128, 4, T, 16)
        return np.ascontiguousarray(arr.transpose(3, 2, 1, 0).reshape(16, T, 512))

    F = unT(hT_fwd)
    Bo = unT(hT_bwd)[:, ::-1, :]
    return np.ascontiguousarray(np.concatenate([F, Bo], axis=2))


def kernel(**inputs):
    from concourse.bass_utils import run_bass_kernel_spmd

    input_ids = np.asarray(inputs["input_ids"])
    T = input_ids.shape[1]
    nc = get_program(T)
    maps = make_in_maps(**inputs)
    res = run_bass_kernel_spmd(nc, maps, list(range(8)))
    return assemble_output(
        res.results[2]["hT_out"], res.results[3]["hT_out"], T
    )


# revision 9
# speedup vs baseline: 1.7991x; 1.0047x over previous
"""BiLSTM Trainium2 kernel v4 (8 NeuronCores, SPMD, pipelined layers).

Roles (selected at runtime from partition id, same program on all cores):
  core 0: fwd layer-0    core 2: fwd layer-1
  core 1: bwd layer-0    core 3: bwd layer-1
  cores 4-7: spare (zero inputs, outputs ignored)

v4 vs v3: the per-step critical chain is restructured around the
engine FIFOs. ScalarE runs exactly the four chain ACTs per step
(tanh g, sigmoid if, sigmoid o, tanh c); the x@Wx bias-add/copy moved
to VectorE (tensor_scalar_add), so bulk-matmul PSUM banks are freed
promptly instead of queueing behind chain ACTs. The cell state c lives
in the right half of a [128,128] tile G whose left half receives
tanh(g), so i*tanh(g) and f*c fuse into ONE VectorE multiply
(igcf = sif * G) followed by one add. x@Wx matmuls are spread across
the recurrence steps (4 N=512 matmuls after each step's o-group) so
they execute inside the PE idle window while the boundary chain runs.

Gate order is permuted host-side to [g, i, f, o]; each gate group
accumulates (on top of an identity-matmul that pre-adds the precomputed
x@Wx+b term) into its own PSUM bank so tanh(g)/sigmoid(i,f) start while
the PE is still accumulating o. h stays transposed everywhere: the
recurrence's weight-stationary matmuls produce gates^T in PSUM at full
128-partition width, h^T feeds the next step directly, and layer-0's h^T
chunks are DMA'd straight to DRAM (no PE transposes anywhere).
"""

import numpy as np
import ml_dtypes

B = 16
H = 512
D = 1024
V = 32000
GATE = 4 * H
CH = 16            # recurrence steps per block
TOKB = CH * B      # tokens per block = 256
SB = 4             # blocks per superblock
SBTOK = SB * TOKB  # tokens per superblock = 1024
LAG = 3            # consumer lag in superblocks
PADB = 256         # hT_out block slots (pow2 so % is cheap); > NB + LAG*SB

_PROGRAM_CACHE = {}


def build_program(T):
    import concourse.mybir as mybir
    import concourse.tile as tile
    from concourse import bacc
    from concourse.bass import ds
    from concourse.masks import make_identity
    from concourse.tile_rust import add_dep_helper

    NT = T * B
    NB = NT // TOKB
    NSB = NB // SB
    NL = NSB + 1 + LAG

    f32 = mybir.dt.float32
    bf16 = mybir.dt.bfloat16
    i16 = mybir.dt.int16
    Sig = mybir.ActivationFunctionType.Sigmoid
    Tnh = mybir.ActivationFunctionType.Tanh

    nc = bacc.Bacc("TRN2", target_bir_lowering=False, debug=True, num_devices=8)

    tbl = nc.declare_dram_parameter("tbl", [V, D], bf16, isOutput=False)
    ids = nc.declare_dram_parameter("ids", [128, T], i16, isOutput=False)
    wx = nc.declare_dram_parameter("wx", [D, GATE], bf16, isOutput=False)
    wh = nc.declare_dram_parameter("wh", [H, GATE], bf16, isOutput=False)
    bt = nc.declare_dram_parameter("bt", [128, 16], f32, isOutput=False)
    hT_out = nc.declare_dram_parameter(
        "hT_out", [128, 4, PADB * TOKB], bf16, isOutput=True
    )

    with tile.TileContext(nc) as tc:
        with (
            tc.tile_pool(name="dram", bufs=1, space="DRAM") as dpool,
            tc.tile_pool(name="consts", bufs=1) as cpool,
            tc.tile_pool(name="xin", bufs=2) as xpool,
            tc.tile_pool(name="gxf", bufs=2) as gxpool,
            tc.tile_pool(name="gxc", bufs=2) as gcpool,
            tc.tile_pool(name="state", bufs=1) as spool,
            tc.tile_pool(name="tmp", bufs=8) as tpool,
            tc.tile_pool(name="hout", bufs=2) as hpool,
            tc.tile_pool(name="ps", bufs=2, space="PSUM") as pspool,
            tc.tile_pool(name="ps2", bufs=2, space="PSUM") as ps2pool,
            tc.tile_pool(name="ps3", bufs=2, space="PSUM") as ps3pool,
            tc.tile_pool(name="psx", bufs=2, space="PSUM") as psxpool,
        ):
            h0x = [dpool.tile([128, 4, SBTOK], bf16, tag=f"h0x{j}", name=f"h0x{j}") for j in range(NSB)]
            h0r = [
                dpool.tile([2, 128, 4, SBTOK], bf16, tag=f"h0r{j}", name=f"h0r{j}")
                for j in range(NSB)
            ]
            gxb = [
                dpool.tile([128, SB, CH, 16, B], bf16, tag=f"gxb{p}", name=f"gxb{p}")
                for p in range(2)
            ]

            pid = nc.partition_id()
            # 0 on layer-0 cores, LAG*SB on layer-1 cores (hT_out block offset)
            ofs_sv = nc.snap(((pid // 2) % 2) * (LAG * SB))

            wx_sb = cpool.tile([128, 8, GATE], bf16, tag="wx")
            nc.sync.dma_start(
                out=wx_sb, in_=wx[:, :].rearrange("(k p) m -> p k m", p=128)
            )
            wh_sb = cpool.tile([128, 4, GATE], bf16, tag="wh")
            nc.sync.dma_start(
                out=wh_sb, in_=wh[:, :].rearrange("(k p) m -> p k m", p=128)
            )
            bt_sb = cpool.tile([128, 16], f32, tag="bt")
            nc.sync.dma_start(out=bt_sb, in_=bt[:, :])
            ids_sb = cpool.tile([128, T], i16, tag="ids")
            nc.sync.dma_start(out=ids_sb, in_=ids[:, :])
            ident = cpool.tile([128, 128], bf16, tag="ident")
            make_identity(nc, ident)

            # G: [tanh(g) scratch | cell state c]
            G_sb = spool.tile([128, 128], f32, tag="G")
            hT_sb = spool.tile([128, 4, 16], bf16, tag="h")
            nc.vector.memset(G_sb, 0.0)
            nc.vector.memset(hT_sb, 0.0)

            # zero the consumed half of the h0r buffers read during pipeline
            # warmup (before any AllGather has filled them)
            zt = cpool.tile([128, 4, SBTOK], bf16, tag="zt")
            nc.vector.memset(zt, 0.0)
            for j in range(min(LAG, NSB)):
                src = (j - LAG) % NSB
                nc.sync.dma_start(out=h0r[src][0], in_=zt)

            colls = {}
            xts = {}
            xmm_state = {}  # t -> [psx, gxf]

            def emit_load(tt):
                """Stage the x-input tile for loop tt (issued two loops early)."""
                xt = xpool.tile([128, 2, 8, 512], bf16, tag="xt")
                xts[tt] = xt
                with tc.If((pid % 4) < 2) as cmp:
                    for g in range(2):
                        nc.gpsimd.dma_gather(
                            xt[:, g, :, :],
                            tbl[:, :],
                            ids_sb[:, ds(((tt % NSB) * SB) * CH + g * 32, 32)],
                            512,
                            512,
                            D,
                            transpose=True,
                        )
                with cmp.Else():
                    nc.vector.memset(xt[:, :, 4:8, :], 0.0)
                    src = (tt - LAG) % NSB
                    d = None
                    for g in range(2):
                        d = nc.sync.dma_start(
                            out=xt[:, g, 0:4, :],
                            in_=h0r[src][0][:, :, g * 512 : (g + 1) * 512],
                        )
                        if 0 <= tt - LAG < NSB and (tt - LAG) in colls:
                            add_dep_helper(
                                d.ins, colls[tt - LAG].ins, reason="xt after allgather"
                            )

            def emit_xmm_mms(t, hs):
                """Half an m-tile (4 of 8 k-matmuls) of x @ Wx for superblock t.
                hs in 0..63 indexes (token-half g, m-tile, k-phase)."""
                g = hs // 32
                m = (hs % 32) // 2
                phase = hs % 2
                xt = xts[t]
                st = xmm_state.setdefault(t, [None, None])
                if hs % 32 == 0:
                    st[1] = gxpool.tile(
                        [128, 2, CH, 16, B], bf16, tag="gxf", name="gxf"
                    )
                if phase == 0:
                    st[0] = psxpool.tile([128, 512], f32, tag="psx", name="psx")
                ps, gxf = st[0], st[1]
                for k in range(4 * phase, 4 * phase + 4):
                    nc.tensor.matmul(
                        ps[:, :],
                        lhsT=wx_sb[:, k, m * 128 : (m + 1) * 128],
                        rhs=xt[:, g, k, :],
                        start=(k == 0),
                        stop=(k == 7),
                    )

            def emit_xmm_tail(t, hs):
                """bias-add + PSUM->SBUF copy on VectorE (emitted after the
                chain DVE ops so it never delays them), + gxb DMA."""
                g = hs // 32
                m = (hs % 32) // 2
                if hs % 2 == 0:
                    return
                ps, gxf = xmm_state[t]
                nc.vector.tensor_scalar_add(
                    gxf[:, :, :, m, :],
                    ps[:, :].rearrange("p (b s c) -> p b s c", b=2, s=CH),
                    bt_sb[:, m : m + 1],
                )
                if m == 15:
                    nc.sync.dma_start(
                        out=gxb[t % 2][:, 2 * g : 2 * g + 2, :, :, :], in_=gxf
                    )

            def emit_rec_body(t, i, with_xmm):
                """Recurrence for block i of superblock t-1 (+ interleaved
                x@Wx pieces of superblock t)."""
                sbi = (t - 1) % NSB
                gxc = gcpool.tile([128, CH, 16, B], bf16, tag="gxc")
                nc.sync.dma_start(out=gxc, in_=gxb[(t - 1) % 2][:, ds(i, 1), :, :, :])
                hTf = hpool.tile([128, 4, CH, 16], bf16, tag="hTf")
                for s in range(CH):
                    # three PSUM banks so early gate groups are readable while
                    # the PE is still accumulating later ones
                    psg_g = pspool.tile([128, 64], f32, tag="psg_g")
                    psg_if = ps2pool.tile([128, 128], f32, tag="psg_if")
                    psg_o = ps3pool.tile([128, 64], f32, tag="psg_o")
                    groups = [
                        (psg_g, 0, 4),
                        (psg_if, 4, 12),
                        (psg_o, 12, 16),
                    ]
                    # ident pre-adds first (adjacent: shared stationary operand)
                    for ptile, m0, m1 in groups:
                        nc.tensor.matmul(
                            ptile[:, :],
                            lhsT=ident,
                            rhs=gxc[:, s, m0:m1, :],
                            start=True,
                            stop=False,
                        )
                    for ptile, m0, m1 in groups:
                        for m in range(m0, m1):
                            for k in range(4):
                                rhs = hT_sb[:, k, :] if s == 0 else hTf[:, k, s - 1, :]
                                nc.tensor.matmul(
                                    ptile[:, (m - m0) * 16 : (m - m0 + 1) * 16],
                                    lhsT=wh_sb[:, k, m * 128 : (m + 1) * 128],
                                    rhs=rhs,
                                    start=False,
                                    stop=(m == m1 - 1 and k == 3),
                                )
                    # PE: x@Wx piece fills the boundary window after o-group
                    if with_xmm:
                        emit_xmm_mms(t, i * CH + s)
                    # ScalarE chain: tg, sif, so, th; VectorE: igcf, cadd, h
                    nc.scalar.activation(G_sb[:, 0:64], psg_g[:, :], Tnh)
                    sif = tpool.tile([128, 128], f32, tag="sif")
                    nc.scalar.activation(sif, psg_if[:, :], Sig)
                    so = tpool.tile([128, 64], f32, tag="so")
                    nc.scalar.activation(so, psg_o[:, :], Sig)
                    igcf = tpool.tile([128, 128], f32, tag="igcf")
                    nc.vector.tensor_mul(igcf, sif, G_sb)
                    nc.vector.tensor_add(G_sb[:, 64:128], igcf[:, 0:64], igcf[:, 64:128])
                    th = tpool.tile([128, 64], f32, tag="th")
                    nc.scalar.activation(th, G_sb[:, 64:128], Tnh)
                    nc.vector.tensor_mul(hTf[:, :, s, :], so, th)
                    if with_xmm:
                        emit_xmm_tail(t, i * CH + s)
                    if s == CH - 1:
                        nc.vector.tensor_copy(hT_sb, hTf[:, :, s, :])
                nc.sync.dma_start(
                    out=h0x[sbi][:, :, ds(i * TOKB, TOKB)], in_=hTf
                )
                goff = ((t - 1) * SB + PADB + i - ofs_sv) % PADB
                nc.sync.dma_start(
                    out=hT_out[:, :, ds(goff * TOKB, TOKB)], in_=hTf
                )

            emit_load(0)
            emit_load(1)
            for t in range(NL):
                with_xmm = t < NL - 1
                for i in range(SB):
                    if t > 0:
                        emit_rec_body(t, i, with_xmm)
                    elif with_xmm:
                        for s in range(CH):
                            emit_xmm_mms(t, i * CH + s)
                            emit_xmm_tail(t, i * CH + s)
                j = t - 1
                if 0 <= j < NSB:
                    colls[j] = nc.gpsimd.collective_compute(
                        "AllGather",
                        mybir.AluOpType.bypass,
                        replica_groups=[[0, 2], [1, 3], [4, 6], [5, 7]],
                        ins=[h0x[j][:]],
                        outs=[h0r[j][:]],
                    )
                if t + 2 < NL:
                    emit_load(t + 2)
                if t == LAG:
                    # layer-1 cores start their real recurrence next loop
                    with tc.If((pid % 4) >= 2):
                        nc.vector.memset(G_sb, 0.0)
                        nc.vector.memset(hT_sb, 0.0)

    nc.finalize()
    return nc


def get_program(T):
    if T not in _PROGRAM_CACHE:
        _PROGRAM_CACHE[T] = build_program(T)
    return _PROGRAM_CACHE[T]


# gate reorder: reference layout [i, g, f, o] -> kernel layout [g, i, f, o]
_PERM = np.r_[512:1024, 0:512, 1024:1536, 1536:2048]


def _prep_weights(W, b, din):
    """W [din+H, 4H], b [4H] -> (wx [D,GATE] bf16, wh [H,GATE] bf16, bt [128,16] f32)."""
    bf = ml_dtypes.bfloat16
    W = np.asarray(W, np.float32)[:, _PERM]
    bv = np.asarray(b, np.float32)[_PERM].copy()
    bv[1024:1536] += 1.0  # haiku forget-gate +1 (f block now at 1024:1536)
    wxp = np.zeros((D, GATE), np.float32)
    wxp[0:din] = W[0:din]
    whp = np.ascontiguousarray(W[din : din + H])
    btp = np.ascontiguousarray(bv.reshape(16, 128).T)
    return wxp.astype(bf), whp.astype(bf), btp


def _wrap_ids(ids2d):
    return np.tile(np.asarray(ids2d).astype(np.int16), (8, 1))


def make_in_maps(input_ids, embed_table, fwd_W0, fwd_b0, fwd_W1, fwd_b1,
                 bwd_W0, bwd_b0, bwd_W1, bwd_b1):
    T = input_ids.shape[1]
    bf = ml_dtypes.bfloat16
    tbl = np.ascontiguousarray(np.asarray(embed_table, np.float32)).astype(bf)

    ids_f = _wrap_ids(input_ids)
    ids_b = _wrap_ids(np.asarray(input_ids)[:, ::-1])

    z = np.zeros
    base = dict(
        tbl=z((V, D), bf),
        ids=z((128, T), np.int16),
        wx=z((D, GATE), bf),
        wh=z((H, GATE), bf),
        bt=z((128, 16), np.float32),
    )
    maps = [dict(base) for _ in range(8)]

    fx0, fh0, fb0t = _prep_weights(fwd_W0, fwd_b0, D)
    bx0, bh0, bb0t = _prep_weights(bwd_W0, bwd_b0, D)
    fx1, fh1, fb1t = _prep_weights(fwd_W1, fwd_b1, H)
    bx1, bh1, bb1t = _prep_weights(bwd_W1, bwd_b1, H)

    maps[0].update(tbl=tbl, ids=ids_f, wx=fx0, wh=fh0, bt=fb0t)
    maps[1].update(tbl=tbl, ids=ids_b, wx=bx0, wh=bh0, bt=bb0t)
    maps[2].update(wx=fx1, wh=fh1, bt=fb1t)
    maps[3].update(wx=bx1, wh=bh1, bt=bb1t)
    return maps


def assemble_output(hT_fwd, hT_bwd, T):
    def unT(a):
        arr = np.asarray(a, np.float32)[:, :, : T * 16].reshape(128, 4, T, 16)
        return np.ascontiguousarray(arr.transpose(3, 2, 1, 0).reshape(16, T, 512))

    F = unT(hT_fwd)
    Bo = unT(hT_bwd)[:, ::-1, :]
    return np.ascontiguousarray(np.concatenate([F, Bo], axis=2))


def kernel(**inputs):
    from concourse.bass_utils import run_bass_kernel_spmd

    input_ids = np.asarray(inputs["input_ids"])
    T = input_ids.shape[1]
    nc = get_program(T)
    maps = make_in_maps(**inputs)
    res = run_bass_kernel_spmd(nc, maps, list(range(8)))
    return assemble_output(
        res.results[2]["hT_out"], res.results[3]["hT_out"], T
    )


# revision 18
# speedup vs baseline: 1.8281x; 1.0161x over previous
"""BiLSTM Trainium2 kernel v4 (8 NeuronCores, SPMD, pipelined layers).

Roles (selected at runtime from partition id, same program on all cores):
  core 0: fwd layer-0    core 2: fwd layer-1
  core 1: bwd layer-0    core 3: bwd layer-1
  cores 4-7: spare (zero inputs, outputs ignored)

v4 vs v3: the per-step critical chain is restructured around the
engine FIFOs. ScalarE runs exactly the four chain ACTs per step
(tanh g, sigmoid if, sigmoid o, tanh c); the x@Wx bias-add/copy moved
to VectorE (tensor_scalar_add), so bulk-matmul PSUM banks are freed
promptly instead of queueing behind chain ACTs. The cell state c lives
in the right half of a [128,128] tile G whose left half receives
tanh(g), so i*tanh(g) and f*c fuse into ONE VectorE multiply
(igcf = sif * G) followed by one add. x@Wx matmuls are spread across
the recurrence steps (4 N=512 matmuls after each step's o-group) so
they execute inside the PE idle window while the boundary chain runs.

Gate order is permuted host-side to [g, i, f, o]; each gate group
accumulates (on top of an identity-matmul that pre-adds the precomputed
x@Wx+b term) into its own PSUM bank so tanh(g)/sigmoid(i,f) start while
the PE is still accumulating o. h stays transposed everywhere: the
recurrence's weight-stationary matmuls produce gates^T in PSUM at full
128-partition width, h^T feeds the next step directly, and layer-0's h^T
chunks are DMA'd straight to DRAM (no PE transposes anywhere).
"""

import numpy as np
import ml_dtypes

B = 16
H = 512
D = 1024
V = 32000
GATE = 4 * H
CH = 16            # recurrence steps per block
TOKB = CH * B      # tokens per block = 256
SB = 4             # blocks per superblock
SBTOK = SB * TOKB  # tokens per superblock = 1024
LAG = 3            # consumer lag in superblocks
PADB = 256         # hT_out block slots (pow2 so % is cheap); > NB + LAG*SB

_PROGRAM_CACHE = {}


def build_program(T):
    import concourse.mybir as mybir
    import concourse.tile as tile
    from concourse import bacc
    from concourse.bass import ds
    from concourse.masks import make_identity
    from concourse.tile_rust import add_dep_helper

    NT = T * B
    NB = NT // TOKB
    NSB = NB // SB
    NL = NSB + 1 + LAG

    f32 = mybir.dt.float32
    bf16 = mybir.dt.bfloat16
    i16 = mybir.dt.int16
    Sig = mybir.ActivationFunctionType.Sigmoid
    Tnh = mybir.ActivationFunctionType.Tanh

    nc = bacc.Bacc("TRN2", target_bir_lowering=False, debug=True, num_devices=8)

    tbl = nc.declare_dram_parameter("tbl", [V, D], bf16, isOutput=False)
    ids = nc.declare_dram_parameter("ids", [128, T], i16, isOutput=False)
    wx = nc.declare_dram_parameter("wx", [D, GATE], bf16, isOutput=False)
    wh = nc.declare_dram_parameter("wh", [H, GATE], bf16, isOutput=False)
    bt = nc.declare_dram_parameter("bt", [128, 16], f32, isOutput=False)
    hT_out = nc.declare_dram_parameter(
        "hT_out", [128, 4, PADB * TOKB], bf16, isOutput=True
    )

    with tile.TileContext(nc) as tc:
        with (
            tc.tile_pool(name="dram", bufs=1, space="DRAM") as dpool,
            tc.tile_pool(name="consts", bufs=1) as cpool,
            tc.tile_pool(name="xin", bufs=2) as xpool,
            tc.tile_pool(name="gxf", bufs=2) as gxpool,
            tc.tile_pool(name="gxc", bufs=2) as gcpool,
            tc.tile_pool(name="state", bufs=1) as spool,
            tc.tile_pool(name="tmp", bufs=8) as tpool,
            tc.tile_pool(name="hout", bufs=2) as hpool,
            tc.tile_pool(name="ps", bufs=2, space="PSUM") as pspool,
            tc.tile_pool(name="ps2", bufs=2, space="PSUM") as ps2pool,
            tc.tile_pool(name="ps3", bufs=2, space="PSUM") as ps3pool,
            tc.tile_pool(name="psx", bufs=2, space="PSUM") as psxpool,
        ):
            h0x = [dpool.tile([128, 4, SBTOK], bf16, tag=f"h0x{j}", name=f"h0x{j}") for j in range(NSB)]
            h0r = [
                dpool.tile([2, 128, 4, SBTOK], bf16, tag=f"h0r{j}", name=f"h0r{j}")
                for j in range(NSB)
            ]
            gxb = [
                dpool.tile([128, SB, CH, 16, B], bf16, tag=f"gxb{p}", name=f"gxb{p}")
                for p in range(2)
            ]

            pid = nc.partition_id()
            # 0 on layer-0 cores, LAG*SB on layer-1 cores (hT_out block offset)
            ofs_sv = nc.snap(((pid // 2) % 2) * (LAG * SB))

            wx_sb = cpool.tile([128, 8, GATE], bf16, tag="wx")
            nc.sync.dma_start(
                out=wx_sb, in_=wx[:, :].rearrange("(k p) m -> p k m", p=128)
            )
            wh_sb = cpool.tile([128, 4, GATE], bf16, tag="wh")
            nc.sync.dma_start(
                out=wh_sb, in_=wh[:, :].rearrange("(k p) m -> p k m", p=128)
            )
            bt_sb = cpool.tile([128, 16], f32, tag="bt")
            nc.sync.dma_start(out=bt_sb, in_=bt[:, :])
            ids_sb = cpool.tile([128, T], i16, tag="ids")
            nc.sync.dma_start(out=ids_sb, in_=ids[:, :])
            ident = cpool.tile([128, 128], bf16, tag="ident")
            make_identity(nc, ident)

            # G: [tanh(g) scratch | cell state c]
            G_sb = spool.tile([128, 128], f32, tag="G")
            hT_sb = spool.tile([128, 4, 16], bf16, tag="h")
            nc.vector.memset(G_sb, 0.0)
            nc.vector.memset(hT_sb, 0.0)

            # zero the consumed half of the h0r buffers read during pipeline
            # warmup (before any AllGather has filled them)
            zt = cpool.tile([128, 4, SBTOK], bf16, tag="zt")
            nc.vector.memset(zt, 0.0)
            for j in range(min(LAG, NSB)):
                src = (j - LAG) % NSB
                nc.sync.dma_start(out=h0r[src][0], in_=zt)

            colls = {}
            xts = {}
            xmm_state = {}  # t -> [psx, gxf]

            def emit_load(tt):
                """Stage the x-input tile for loop tt (issued two loops early)."""
                xt = xpool.tile([128, 2, 8, 512], bf16, tag="xt")
                xts[tt] = xt
                with tc.If((pid % 4) < 2) as cmp:
                    for g in range(2):
                        nc.gpsimd.dma_gather(
                            xt[:, g, :, :],
                            tbl[:, :],
                            ids_sb[:, ds(((tt % NSB) * SB) * CH + g * 32, 32)],
                            512,
                            512,
                            D,
                            transpose=True,
                        )
                with cmp.Else():
                    nc.vector.memset(xt[:, :, 4:8, :], 0.0)
                    src = (tt - LAG) % NSB
                    d = None
                    for g in range(2):
                        d = nc.sync.dma_start(
                            out=xt[:, g, 0:4, :],
                            in_=h0r[src][0][:, :, g * 512 : (g + 1) * 512],
                        )
                        if 0 <= tt - LAG < NSB and (tt - LAG) in colls:
                            add_dep_helper(
                                d.ins, colls[tt - LAG].ins, reason="xt after allgather"
                            )

            def emit_xmm_mms(t, hs):
                """Half an m-tile (4 of 8 k-matmuls) of x @ Wx for superblock t.
                hs in 0..63 indexes (token-half g, m-tile, k-phase)."""
                g = hs // 32
                m = (hs % 32) // 2
                phase = hs % 2
                xt = xts[t]
                st = xmm_state.setdefault(t, [None, None])
                if hs % 32 == 0:
                    st[1] = gxpool.tile(
                        [128, 2, CH, 16, B], bf16, tag="gxf", name="gxf"
                    )
                if phase == 0:
                    st[0] = psxpool.tile([128, 512], f32, tag="psx", name="psx")
                ps, gxf = st[0], st[1]
                for k in range(4 * phase, 4 * phase + 4):
                    nc.tensor.matmul(
                        ps[:, :],
                        lhsT=wx_sb[:, k, m * 128 : (m + 1) * 128],
                        rhs=xt[:, g, k, :],
                        start=(k == 0),
                        stop=(k == 7),
                    )

            def emit_xmm_tail(t, hs):
                """bias-add + PSUM->SBUF copy on VectorE (emitted after the
                chain DVE ops so it never delays them), + gxb DMA."""
                g = hs // 32
                m = (hs % 32) // 2
                if hs % 2 == 0:
                    return
                ps, gxf = xmm_state[t]
                nc.vector.tensor_scalar_add(
                    gxf[:, :, :, m, :],
                    ps[:, :].rearrange("p (b s c) -> p b s c", b=2, s=CH),
                    bt_sb[:, m : m + 1],
                )
                if m == 15:
                    nc.sync.dma_start(
                        out=gxb[t % 2][:, 2 * g : 2 * g + 2, :, :, :], in_=gxf
                    )

            def emit_rec_body(t, i, with_xmm):
                """Recurrence for block i of superblock t-1 (+ interleaved
                x@Wx pieces of superblock t)."""
                sbi = (t - 1) % NSB
                gxc = gcpool.tile([128, CH, 16, B], bf16, tag="gxc")
                nc.sync.dma_start(out=gxc, in_=gxb[(t - 1) % 2][:, ds(i, 1), :, :, :])
                hTf = hpool.tile([128, 4, CH, 16], bf16, tag="hTf")
                for s in range(CH):
                    # three PSUM banks so early gate groups are readable while
                    # the PE is still accumulating later ones
                    psg_g = pspool.tile([128, 64], f32, tag="psg_g")
                    psg_if = ps2pool.tile([128, 128], f32, tag="psg_if")
                    psg_o = ps3pool.tile([128, 64], f32, tag="psg_o")
                    groups = [
                        (psg_if, 4, 12),
                        (psg_g, 0, 4),
                        (psg_o, 12, 16),
                    ]
                    # ident pre-adds first (adjacent: shared stationary operand)
                    for ptile, m0, m1 in groups:
                        nc.tensor.matmul(
                            ptile[:, :],
                            lhsT=ident,
                            rhs=gxc[:, s, m0:m1, :],
                            start=True,
                            stop=False,
                        )
                    for ptile, m0, m1 in groups:
                        for m in range(m0, m1):
                            for k in range(4):
                                rhs = hT_sb[:, k, :] if s == 0 else hTf[:, k, s - 1, :]
                                nc.tensor.matmul(
                                    ptile[:, (m - m0) * 16 : (m - m0 + 1) * 16],
                                    lhsT=wh_sb[:, k, m * 128 : (m + 1) * 128],
                                    rhs=rhs,
                                    start=False,
                                    stop=(m == m1 - 1 and k == 3),
                                )
                    # PE: x@Wx piece fills the boundary window after o-group
                    if with_xmm:
                        emit_xmm_mms(t, i * CH + s)
                    # ScalarE chain: tg, sif, so, th; VectorE: igcf, cadd, h
                    sif = tpool.tile([128, 128], f32, tag="sif")
                    nc.scalar.activation(sif, psg_if[:, :], Sig)
                    nc.scalar.activation(G_sb[:, 0:64], psg_g[:, :], Tnh)
                    so = tpool.tile([128, 64], f32, tag="so")
                    nc.scalar.activation(so, psg_o[:, :], Sig)
                    igcf = tpool.tile([128, 128], f32, tag="igcf")
                    nc.vector.tensor_mul(igcf, sif, G_sb)
                    nc.vector.tensor_add(G_sb[:, 64:128], igcf[:, 0:64], igcf[:, 64:128])
                    th = tpool.tile([128, 64], f32, tag="th")
                    nc.scalar.activation(th, G_sb[:, 64:128], Tnh)
                    nc.vector.tensor_mul(hTf[:, :, s, :], so, th)
                    if with_xmm:
                        emit_xmm_tail(t, i * CH + s)
                    if s == CH - 1:
                        nc.vector.tensor_copy(hT_sb, hTf[:, :, s, :])
                nc.sync.dma_start(
                    out=h0x[sbi][:, :, ds(i * TOKB, TOKB)], in_=hTf
                )
                goff = ((t - 1) * SB + PADB + i - ofs_sv) % PADB
                nc.sync.dma_start(
                    out=hT_out[:, :, ds(goff * TOKB, TOKB)], in_=hTf
                )

            emit_load(0)
            emit_load(1)
            for t in range(NL):
                with_xmm = t < NL - 1
                for i in range(SB):
                    if t > 0:
                        emit_rec_body(t, i, with_xmm)
                    elif with_xmm:
                        for s in range(CH):
                            emit_xmm_mms(t, i * CH + s)
                            emit_xmm_tail(t, i * CH + s)
                j = t - 1
                if 0 <= j < NSB:
                    colls[j] = nc.gpsimd.collective_compute(
                        "AllGather",
                        mybir.AluOpType.bypass,
                        replica_groups=[[0, 2], [1, 3], [4, 6], [5, 7]],
                        ins=[h0x[j][:]],
                        outs=[h0r[j][:]],
                    )
                if t + 2 < NL:
                    emit_load(t + 2)
                if t == LAG:
                    # layer-1 cores start their real recurrence next loop
                    with tc.If((pid % 4) >= 2):
                        nc.vector.memset(G_sb, 0.0)
                        nc.vector.memset(hT_sb, 0.0)

    nc.finalize()
    return nc


def get_program(T):
    if T not in _PROGRAM_CACHE:
        _PROGRAM_CACHE[T] = build_program(T)
    return _PROGRAM_CACHE[T]


# gate reorder: reference layout [i, g, f, o] -> kernel layout [g, i, f, o]
_PERM = np.r_[512:1024, 0:512, 1024:1536, 1536:2048]


def _prep_weights(W, b, din):
    """W [din+H, 4H], b [4H] -> (wx [D,GATE] bf16, wh [H,GATE] bf16, bt [128,16] f32)."""
    bf = ml_dtypes.bfloat16
    W = np.asarray(W, np.float32)[:, _PERM]
    bv = np.asarray(b, np.float32)[_PERM].copy()
    bv[1024:1536] += 1.0  # haiku forget-gate +1 (f block now at 1024:1536)
    wxp = np.zeros((D, GATE), np.float32)
    wxp[0:din] = W[0:din]
    whp = np.ascontiguousarray(W[din : din + H])
    btp = np.ascontiguousarray(bv.reshape(16, 128).T)
    return wxp.astype(bf), whp.astype(bf), btp


def _wrap_ids(ids2d):
    return np.tile(np.asarray(ids2d).astype(np.int16), (8, 1))


def make_in_maps(input_ids, embed_table, fwd_W0, fwd_b0, fwd_W1, fwd_b1,
                 bwd_W0, bwd_b0, bwd_W1, bwd_b1):
    T = input_ids.shape[1]
    bf = ml_dtypes.bfloat16
    tbl = np.ascontiguousarray(np.asarray(embed_table, np.float32)).astype(bf)

    ids_f = _wrap_ids(input_ids)
    ids_b = _wrap_ids(np.asarray(input_ids)[:, ::-1])

    z = np.zeros
    base = dict(
        tbl=z((V, D), bf),
        ids=z((128, T), np.int16),
        wx=z((D, GATE), bf),
        wh=z((H, GATE), bf),
        bt=z((128, 16), np.float32),
    )
    maps = [dict(base) for _ in range(8)]

    fx0, fh0, fb0t = _prep_weights(fwd_W0, fwd_b0, D)
    bx0, bh0, bb0t = _prep_weights(bwd_W0, bwd_b0, D)
    fx1, fh1, fb1t = _prep_weights(fwd_W1, fwd_b1, H)
    bx1, bh1, bb1t = _prep_weights(bwd_W1, bwd_b1, H)

    maps[0].update(tbl=tbl, ids=ids_f, wx=fx0, wh=fh0, bt=fb0t)
    maps[1].update(tbl=tbl, ids=ids_b, wx=bx0, wh=bh0, bt=bb0t)
    maps[2].update(wx=fx1, wh=fh1, bt=fb1t)
    maps[3].update(wx=bx1, wh=bh1, bt=bb1t)
    return maps


def assemble_output(hT_fwd, hT_bwd, T):
    def unT(a):
        arr = np.asarray(a, np.float32)[:, :, : T * 16].reshape(128, 4, T, 16)
        return np.ascontiguousarray(arr.transpose(3, 2, 1, 0).reshape(16, T, 512))

    F = unT(hT_fwd)
    Bo = unT(hT_bwd)[:, ::-1, :]
    return np.ascontiguousarray(np.concatenate([F, Bo], axis=2))


def kernel(**inputs):
    from concourse.bass_utils import run_bass_kernel_spmd

    input_ids = np.asarray(inputs["input_ids"])
    T = input_ids.shape[1]
    nc = get_program(T)
    maps = make_in_maps(**inputs)
    res = run_bass_kernel_spmd(nc, maps, list(range(8)))
    return assemble_output(
        res.results[2]["hT_out"], res.results[3]["hT_out"], T
    )
